# revision 14
# baseline (speedup 1.0000x reference)
"""Trainium2 Bass kernel for nn_Attention_41085657153620.

Reference (per batch b):
    e[i,j] = (q_i * w3) @ k_j + q_i @ w1 + k_j @ w2 + bias
    v      = softmax(e, axis=-1) @ k

Key algebraic reduction: the softmax over j is invariant to the
row-constant terms (q_i @ w1 + bias), so only
    s[i,j] = (q_i * w3) @ k_j + ek_j        with ek = k @ w2
matters. Scores are small (|s| < ~5 for this input distribution), so no
max-subtraction is needed before exp.

Layout strategy (one batch per NeuronCore, 8 cores):
  - Everything runs in bf16 on the PE (1 cyc/col, same peak as f32r,
    but transposes are 2x faster than fp32 and LDWEIGHTS halves).
    Measured end-to-end relative error ~5e-3 vs the 2e-2 gate.
  - Scores are computed TRANSPOSED: S^T[j, i] = sum_d kT[d,j] qsT[d,i],
    so the exp'd score tiles are directly usable as the stationary
    (lhsT) operand of the A @ K matmul -- no transpose of A needed.
  - ek_j is folded into pass 1 of the FIRST 256-row sub-block as an
    extra moving column (w2 appended to qsT), then cached in SBUF and
    applied as the exp's per-partition ACT bias for every block. This
    removes the 42us of 1-column fp32 matmuls the old kernel spent.
  - w3 is folded into q on the DVE (q * w3bc) before the PE transpose,
    with w3 pre-broadcast host-side to [128, 512].
  - The four [128,128] transposes of each 128-row group go into ONE
    bf16 psum tile and are evacuated by ONE strided ACT copy, instead
    of per-tile copies that used to stall the transpose chain.
  - The softmax denominator comes from ones-columns appended to the AV
    rhs (kr chunk layout: [k[:,:256] | 1 | 0 | k[:,256:] | 1 | 0]), so
    each accumulated AV psum tile carries sum_j exp(s) in column 256.
    Division happens once per 128 output rows (DVE reciprocal + ACT
    scale).
  - Pass 1 uses 512-wide moving operands (a full psum bank) to halve
    instruction-issue overhead; block 0 is split into 257/256-wide
    sub-blocks to make room for the ek column.

The walrus build in this container refuses any instruction carrying
more than one sync wait (the TRN2 ISA has a single wait slot), so after
Tile scheduling we split multi-wait instructions into single-wait
EventSemaphore carriers (split_multi_waits below).
"""

import ml_dtypes
import numpy as np

import bass_rust
import concourse.bass as bass
import concourse.mybir as mybir
from concourse.bass_utils import run_bass_kernel_spmd
from concourse.tile import TileContext

F32 = mybir.dt.float32
BF16 = mybir.dt.bfloat16
AF = mybir.ActivationFunctionType

B, QL, KL, D = 8, 4096, 4096, 512
BQ = 512                 # q rows per block
NBLK = QL // BQ          # 8
NC = KL // 128           # 32 j-chunks
DC = D // 128            # 4 d-chunks
NIH = BQ // 128          # output row-slices per block (4)
N_CORES = 8


def split_multi_waits(nc):
    """Rewrite instructions with >1 sync wait into single-wait form."""
    n_split = 0
    for f in nc.m.functions:
        for blk in f.blocks:
            insts = list(blk.instructions)
            out = []
            changed = False
            for inst in insts:
                si = inst.sync_info
                if si is not None and len(si.on_wait) > 1:
                    waits = list(si.on_wait)
                    ups = list(si.on_update)
                    assert len(ups) <= 1, (inst.name, ups)
                    for w in waits[:-1]:
                        carrier = mybir.InstEventSemaphore(
                            name=nc.get_next_instruction_name(), ins=[], outs=[]
                        )
                        carrier.engine = inst.engine
                        carrier.sync_info = bass_rust.SyncInfo(
                            on_wait=[w], on_update=[]
                        )
                        nc.register_instruction(carrier, overwrite=True)
                        out.append(carrier)
                        n_split += 1
                    inst.sync_info = bass_rust.SyncInfo(
                        on_wait=[waits[-1]], on_update=ups
                    )
                    changed = True
                out.append(inst)
            if changed:
                blk.instructions = out
    return n_split


def build_attention_nc(reps=1):
    """reps>1 repeats the whole computation in one NEFF (timing only)."""
    nc = bass.Bass()
    q = nc.dram_tensor("q", [QL, D], F32, kind="ExternalInput")
    k = nc.dram_tensor("k", [KL, D], F32, kind="ExternalInput")
    w3bc = nc.dram_tensor("w3bc", [128, D], F32, kind="ExternalInput")
    w2c16 = nc.dram_tensor("w2c16", [128, DC], BF16, kind="ExternalInput")
    id16 = nc.dram_tensor("id16", [128, 128], BF16, kind="ExternalInput")
    v = nc.dram_tensor("v", [QL, D], F32, kind="ExternalOutput")

    with TileContext(nc) as tc:
        with (
            tc.tile_pool(name="const", bufs=1) as const,
            tc.tile_pool(name="stage", bufs=3) as stage,
            tc.tile_pool(name="qstp", bufs=2) as qstp,
            tc.tile_pool(name="qpool", bufs=2) as qpool,
            tc.tile_pool(name="expp", bufs=2) as expp,
            tc.tile_pool(name="outp", bufs=2) as outp,
            tc.tile_pool(name="psT", bufs=2, space="PSUM") as psT,
            tc.tile_pool(name="psS", bufs=2, space="PSUM") as psS,
            tc.tile_pool(name="psO", bufs=2, space="PSUM") as psO,
        ):
            for _rep in range(reps):
                # ---- constants (scalar hwdge queue: gpsimd soft-DGE drains
                # cost ~1us each at startup) -----------------------------------
                w3sb = const.tile([128, D], F32, tag="w3sb")
                identf = const.tile([128, 128], BF16, tag="identf")
                nc.scalar.dma_start(identf[:], id16[:, :])
                nc.scalar.dma_start(w3sb[:], w3bc[:, :])

                # kTr: d-major K (stationary operand of the S^T matmul)
                kTr = const.tile([128, DC, KL], BF16, tag="kTr")
                # kr: j-major K augmented with ones/pad columns (AV rhs).
                # Layout per chunk: [k[:, 0:256] | 1 | 0 0 0 | k[:, 256:512]
                # | 1 | 0 0 0] so that the four 128-col d-slices used as
                # transpose inputs all start 8B-aligned (offsets 0/256/520/
                # 776 bytes) and the two AV rhs slices are [0:258], [260:518].
                kr = const.tile([128, NC, 520], BF16, tag="kr")
                # ek = k @ w2, one column per j-chunk (exp bias), f32
                ek_sb = const.tile([128, NC], F32, tag="ek_sb")
                # block-0 qsT tiles (257-col sub-block carries the w2 column)
                qsT0 = const.tile([128, DC, 260], BF16, tag="qsT0")
                qsT1 = const.tile([128, DC, 256], BF16, tag="qsT1")
                nc.scalar.dma_start(qsT0[:, :, 256:257], w2c16[:, :])

                # prefetch q block 0 ahead of the k chunk stream; per-t DMAs so
                # the first qsb mul starts after 256KB instead of 1MB
                def q_fetch(i0):
                    qt = qstp.tile([128, NIH, D], F32, tag="qst")
                    for t in range(NIH):
                        nc.sync.dma_start(
                            qt[:, t, :], q[i0 + t * 128:i0 + (t + 1) * 128, :]
                        )
                    return qt

                qst_next = q_fetch(0)

                # ones column of kr (softmax denominator), once, strided
                # across all chunks; pads keep the d-slices 8B-aligned
                nc.gpsimd.memset(kr[:, :, 256:257], 1.0)
                nc.gpsimd.memset(kr[:, :, 257:260], 0.0)

                # ---- block-0 q prep (before the k stream so the PE can start
                # pass 1 the moment the first k chunks land) --------------------
                qst = qst_next
                qsb = qpool.tile([128, NIH, D], BF16, tag="qsb")
                for t in range(NIH):
                    nc.vector.tensor_mul(qsb[:, t, :], qst[:, t, :], w3sb[:])
                for (dst, t, col) in [
                    (qsT0, 0, 0), (qsT0, 1, 128), (qsT1, 2, 0), (qsT1, 3, 128),
                ]:
                    pt = psT.tile([128, DC, 128], BF16, tag="psT")
                    for dc in range(DC):
                        nc.tensor.transpose(
                            pt[:, dc, :], qsb[:, t, dc * 128:(dc + 1) * 128],
                            identf[:],
                        )
                    nc.vector.tensor_copy(dst[:, :, col:col + 128], pt[:])

                # ---- merged k setup + block-0 pass 1, software-pipelined by
                # one chunk: per chunk the PE does 4 transposes + 8 matmuls,
                # the DVE does evac(c-1) + casts(c) + ek(c-1), the ACT does the
                # two exps of chunk c-1. All engine budgets sit under the PE's
                # ~1.35us, so the PE never starves after the first chunk. -----
                expT0 = expp.tile([128, NC, BQ], BF16, tag="expT")
                ktiles = {}

                def k_stage(c):
                    kst = stage.tile([128, D], F32, tag="kst")
                    eng = nc.scalar if (c % 2) else nc.sync
                    eng.dma_start(kst[:], k[c * 128:(c + 1) * 128, :])
                    nc.vector.tensor_copy(kr[:, c, 0:256], kst[:, 0:256])
                    nc.vector.tensor_copy(kr[:, c, 260:516], kst[:, 256:512])
                    pt = psT.tile([128, DC, 128], BF16, tag="psT")
                    ksl = [
                        kr[:, c, 0:128], kr[:, c, 128:256],
                        kr[:, c, 260:388], kr[:, c, 388:516],
                    ]
                    for dc in range(DC):
                        nc.tensor.transpose(pt[:, dc, :], ksl[dc], identf[:])
                    ktiles[c] = pt

                def k_evac(c):
                    nc.vector.tensor_copy(
                        kTr[:, :, c * 128:(c + 1) * 128], ktiles.pop(c)[:]
                    )

                def p1_blk0(c):
                    ps_s = psS.tile([128, BQ], F32, tag="psS")
                    for dc in range(DC):
                        nc.tensor.matmul(
                            ps_s[:, 0:257],
                            kTr[:, dc, c * 128:(c + 1) * 128],
                            qsT0[:, dc, 0:257],
                            start=(dc == 0), stop=(dc == DC - 1),
                        )
                    nc.vector.tensor_copy(ek_sb[:, c:c + 1], ps_s[:, 256:257])
                    nc.scalar.activation(
                        expT0[:, c, 0:256], ps_s[:, 0:256], AF.Exp,
                        bias=ek_sb[:, c:c + 1], scale=1.0,
                    )
                    ps_s2 = psS.tile([128, BQ], F32, tag="psS")
                    for dc in range(DC):
                        nc.tensor.matmul(
                            ps_s2[:, 0:256],
                            kTr[:, dc, c * 128:(c + 1) * 128],
                            qsT1[:, dc, 0:256],
                            start=(dc == 0), stop=(dc == DC - 1),
                        )
                    nc.scalar.activation(
                        expT0[:, c, 256:512], ps_s2[:, 0:256], AF.Exp,
                        bias=ek_sb[:, c:c + 1], scale=1.0,
                    )

                k_stage(0)
                for c in range(NC):
                    if c + 1 < NC:
                        k_stage(c + 1)
                    k_evac(c)
                    p1_blk0(c)

                # ---- main loop over q blocks ----------------------------------
                for blk in range(NBLK):
                    i0 = blk * BQ
                    qst = qst_next
                    if blk + 1 < NBLK:
                        qst_next = qstp.tile([128, NIH, D], F32, tag="qst")
                        nc.sync.dma_start(
                            qst_next[:],
                            q[i0 + BQ:i0 + 2 * BQ, :].rearrange(
                                "(t p) d -> p t d", p=128
                            ),
                        )
                    if blk == 0:
                        expT = expT0
                    else:
                        # qsb = bf16(q * w3), then transpose to d-major qsT
                        qsb = qpool.tile([128, NIH, D], BF16, tag="qsb")
                        for t in range(NIH):
                            nc.vector.tensor_mul(
                                qsb[:, t, :], qst[:, t, :], w3sb[:]
                            )
                        qsTn = qpool.tile([128, DC, BQ], BF16, tag="qsTn")
                        for t in range(NIH):
                            pt = psT.tile([128, DC, 128], BF16, tag="psT")
                            for dc in range(DC):
                                nc.tensor.transpose(
                                    pt[:, dc, :],
                                    qsb[:, t, dc * 128:(dc + 1) * 128],
                                    identf[:],
                                )
                            nc.vector.tensor_copy(
                                qsTn[:, :, t * 128:(t + 1) * 128], pt[:]
                            )

                        # pass 1: S^T = kT.T @ qsT chunk by chunk; exp into expT
                        expT = expp.tile([128, NC, BQ], BF16, tag="expT")
                        for c in range(NC):
                            ps_s = psS.tile([128, BQ], F32, tag="psS")
                            for dc in range(DC):
                                nc.tensor.matmul(
                                    ps_s[:],
                                    kTr[:, dc, c * 128:(c + 1) * 128],
                                    qsTn[:, dc, :],
                                    start=(dc == 0),
                                    stop=(dc == DC - 1),
                                )
                            nc.scalar.activation(
                                expT[:, c, :], ps_s[:], AF.Exp,
                                bias=ek_sb[:, c:c + 1], scale=1.0,
                            )

                    # pass 2: AV accumulation per 128-row output slice
                    for ih in range(NIH):
                        pA = psO.tile([128, 257], F32, tag="pA")
                        pB = psO.tile([128, 256], F32, tag="pB")
                        for c in range(NC):
                            lhsT = expT[:, c, ih * 128:(ih + 1) * 128]
                            nc.tensor.matmul(
                                pA[:], lhsT, kr[:, c, 0:257],
                                start=(c == 0), stop=(c == NC - 1),
                            )
                            nc.tensor.matmul(
                                pB[:], lhsT, kr[:, c, 260:516],
                                start=(c == 0), stop=(c == NC - 1),
                            )
                        rec = outp.tile([128, 1], F32, tag="rec")
                        nc.vector.reciprocal(rec[:], pA[:, 256:257])
                        osb = outp.tile([128, 512], F32, tag="osb")
                        nc.scalar.activation(
                            osb[:, 0:256], pA[:, 0:256], AF.Copy, scale=rec[:]
                        )
                        nc.scalar.activation(
                            osb[:, 256:512], pB[:, 0:256], AF.Copy, scale=rec[:]
                        )
                        # two half-width out DMAs so the first half streams
                        # while the second half normalizes (trims the tail)
                        nc.sync.dma_start(
                            v[i0 + ih * 128:i0 + (ih + 1) * 128, 0:256],
                            osb[:, 0:256],
                        )
                        nc.sync.dma_start(
                            v[i0 + ih * 128:i0 + (ih + 1) * 128, 256:512],
                            osb[:, 256:512],
                        )

    split_multi_waits(nc)
    return nc


_NC_CACHE = None


def _get_nc():
    global _NC_CACHE
    if _NC_CACHE is None:
        _NC_CACHE = build_attention_nc()
    return _NC_CACHE


def _host_consts(W):
    w2 = np.ascontiguousarray(W[D:2 * D, 0])
    w3 = np.ascontiguousarray(W[2 * D:3 * D, 0])
    w3bc = np.broadcast_to(w3[None, :], (128, D)).copy()
    # w2 striped the way the d-major transpose lays q out: [p, dc]
    w2c16 = w2.reshape(DC, 128).T.astype(ml_dtypes.bfloat16).copy()
    id16 = np.eye(128, dtype=ml_dtypes.bfloat16)
    return w3bc, w2c16, id16


def run(q, k, W, b, trace=False, **spmd_kwargs):
    nc = _get_nc()
    w3bc, w2c16, id16 = _host_consts(np.asarray(W))
    in_maps = [
        {
            "q": np.ascontiguousarray(q[c]),
            "k": np.ascontiguousarray(k[c]),
            "w3bc": w3bc,
            "w2c16": w2c16,
            "id16": id16,
        }
        for c in range(N_CORES)
    ]
    res = run_bass_kernel_spmd(
        nc, in_maps, list(range(N_CORES)), trace=trace, **spmd_kwargs
    )
    out = np.stack([res.results[c]["v"] for c in range(N_CORES)], axis=0)
    return out, res


def kernel(q, k, W, b):
    out, _ = run(np.asarray(q), np.asarray(k), np.asarray(W), np.asarray(b))
    return out


# revision 18
# speedup vs baseline: 1.0172x; 1.0172x over previous
"""Trainium2 Bass kernel for nn_Attention_41085657153620.

Reference (per batch b):
    e[i,j] = (q_i * w3) @ k_j + q_i @ w1 + k_j @ w2 + bias
    v      = softmax(e, axis=-1) @ k

Key algebraic reduction: the softmax over j is invariant to the
row-constant terms (q_i @ w1 + bias), so only
    s[i,j] = (q_i * w3) @ k_j + ek_j        with ek = k @ w2
matters. Scores are small (|s| < ~5 for this input distribution), so no
max-subtraction is needed before exp.

Layout strategy (one batch per NeuronCore, 8 cores):
  - Everything runs in bf16 on the PE (1 cyc/col, same peak as f32r,
    but transposes are 2x faster than fp32 and LDWEIGHTS halves).
    Measured end-to-end relative error ~5e-3 vs the 2e-2 gate.
  - Scores are computed TRANSPOSED: S^T[j, i] = sum_d kT[d,j] qsT[d,i],
    so the exp'd score tiles are directly usable as the stationary
    (lhsT) operand of the A @ K matmul -- no transpose of A needed.
  - ek_j is folded into pass 1 of the FIRST 256-row sub-block as an
    extra moving column (w2 appended to qsT), then cached in SBUF and
    applied as the exp's per-partition ACT bias for every block. This
    removes the 42us of 1-column fp32 matmuls the old kernel spent.
  - w3 is folded into q on the DVE (q * w3bc) before the PE transpose,
    with w3 pre-broadcast host-side to [128, 512].
  - The four [128,128] transposes of each 128-row group go into ONE
    bf16 psum tile and are evacuated by ONE strided ACT copy, instead
    of per-tile copies that used to stall the transpose chain.
  - The softmax denominator comes from a ones-column appended to the AV
    rhs (kr chunk layout: [k[:,:256] | 1 | pad | k[:,256:]]), so the
    first AV psum tile carries sum_j exp(s) in column 256. Division
    happens once per 128 output rows (DVE reciprocal + ACT scale).
  - Pass 1 uses 512-wide moving operands (a full psum bank) to halve
    instruction-issue overhead; block 0 is split into 257/256-wide
    sub-blocks to make room for the ek column.

The walrus build in this container refuses any instruction carrying
more than one sync wait (the TRN2 ISA has a single wait slot), so after
Tile scheduling we split multi-wait instructions into single-wait
EventSemaphore carriers (split_multi_waits below).
"""

import ml_dtypes
import numpy as np

import bass_rust
import concourse.bass as bass
import concourse.mybir as mybir
from concourse.bass_utils import run_bass_kernel_spmd
from concourse.tile import TileContext

F32 = mybir.dt.float32
BF16 = mybir.dt.bfloat16
AF = mybir.ActivationFunctionType

B, QL, KL, D = 8, 4096, 4096, 512
BQ = 512                 # q rows per block
NBLK = QL // BQ          # 8
NC = KL // 128           # 32 j-chunks
DC = D // 128            # 4 d-chunks
NIH = BQ // 128          # output row-slices per block (4)
N_CORES = 8


def split_multi_waits(nc):
    """Rewrite instructions with >1 sync wait into single-wait form."""
    n_split = 0
    for f in nc.m.functions:
        for blk in f.blocks:
            insts = list(blk.instructions)
            out = []
            changed = False
            for inst in insts:
                si = inst.sync_info
                if si is not None and len(si.on_wait) > 1:
                    waits = list(si.on_wait)
                    ups = list(si.on_update)
                    assert len(ups) <= 1, (inst.name, ups)
                    for w in waits[:-1]:
                        carrier = mybir.InstEventSemaphore(
                            name=nc.get_next_instruction_name(), ins=[], outs=[]
                        )
                        carrier.engine = inst.engine
                        carrier.sync_info = bass_rust.SyncInfo(
                            on_wait=[w], on_update=[]
                        )
                        nc.register_instruction(carrier, overwrite=True)
                        out.append(carrier)
                        n_split += 1
                    inst.sync_info = bass_rust.SyncInfo(
                        on_wait=[waits[-1]], on_update=ups
                    )
                    changed = True
                out.append(inst)
            if changed:
                blk.instructions = out
    return n_split


def build_attention_nc(reps=1):
    """reps>1 repeats the whole computation in one NEFF (timing only)."""
    nc = bass.Bass()
    q = nc.dram_tensor("q", [QL, D], F32, kind="ExternalInput")
    k = nc.dram_tensor("k", [KL, D], F32, kind="ExternalInput")
    w3bc = nc.dram_tensor("w3bc", [128, D], F32, kind="ExternalInput")
    w2c16 = nc.dram_tensor("w2c16", [128, DC], BF16, kind="ExternalInput")
    id16 = nc.dram_tensor("id16", [128, 128], BF16, kind="ExternalInput")
    v = nc.dram_tensor("v", [QL, D], F32, kind="ExternalOutput")

    with TileContext(nc) as tc:
        with (
            tc.tile_pool(name="const", bufs=1) as const,
            tc.tile_pool(name="stage", bufs=3) as stage,
            tc.tile_pool(name="qstp", bufs=2) as qstp,
            tc.tile_pool(name="qpool", bufs=2) as qpool,
            tc.tile_pool(name="expp", bufs=2) as expp,
            tc.tile_pool(name="outp", bufs=2) as outp,
            tc.tile_pool(name="psT", bufs=2, space="PSUM") as psT,
            tc.tile_pool(name="psS", bufs=2, space="PSUM") as psS,
            tc.tile_pool(name="psO", bufs=2, space="PSUM") as psO,
        ):
            for _rep in range(reps):
                # ---- constants (scalar hwdge queue: gpsimd soft-DGE drains
                # cost ~1us each at startup) -----------------------------------
                w3sb = const.tile([128, D], F32, tag="w3sb")
                identf = const.tile([128, 128], BF16, tag="identf")
                nc.scalar.dma_start(identf[:], id16[:, :])
                nc.scalar.dma_start(w3sb[:], w3bc[:, :])

                # kTr: d-major K (stationary operand of the S^T matmul)
                kTr = const.tile([128, DC, KL], BF16, tag="kTr")
                # kr: j-major K augmented with a ones column (AV rhs).
                # Layout per chunk: [k[:, 0:256] | 1 | 0 0 0 | k[:, 256:512]]
                # so that the four 128-col d-slices used as transpose inputs
                # all start 8B-aligned (offsets 0/256/520/776 bytes) and the
                # two AV rhs slices are [0:257] (with denominator), [260:516].
                kr = const.tile([128, NC, 520], BF16, tag="kr")
                # ek = k @ w2, one column per j-chunk (exp bias), f32
                ek_sb = const.tile([128, NC], F32, tag="ek_sb")
                # block-0 qsT tiles (257-col sub-block carries the w2 column)
                qsT0 = const.tile([128, DC, 260], BF16, tag="qsT0")
                qsT1 = const.tile([128, DC, 256], BF16, tag="qsT1")
                nc.scalar.dma_start(qsT0[:, :, 256:257], w2c16[:, :])

                # prefetch q block 0 ahead of the k chunk stream
                qst_next = qstp.tile([128, NIH, D], F32, tag="qst")
                nc.sync.dma_start(
                    qst_next[:], q[0:BQ, :].rearrange("(t p) d -> p t d", p=128)
                )

                # ones column of kr (softmax denominator), once, strided
                # across all chunks; pads keep the d-slices 8B-aligned
                nc.gpsimd.memset(kr[:, :, 256:257], 1.0)
                nc.gpsimd.memset(kr[:, :, 257:260], 0.0)

                # ---- block-0 q prep (before the k stream so the PE can start
                # pass 1 the moment the first k chunks land) --------------------
                qst = qst_next
                qsb = qpool.tile([128, NIH, D], BF16, tag="qsb")
                for t in range(NIH):
                    nc.vector.tensor_mul(qsb[:, t, :], qst[:, t, :], w3sb[:])
                for (dst, t, col) in [
                    (qsT0, 0, 0), (qsT0, 1, 128), (qsT1, 2, 0), (qsT1, 3, 128),
                ]:
                    pt = psT.tile([128, DC, 128], BF16, tag="psT")
                    for dc in range(DC):
                        nc.tensor.transpose(
                            pt[:, dc, :], qsb[:, t, dc * 128:(dc + 1) * 128],
                            identf[:],
                        )
                    nc.vector.tensor_copy(dst[:, :, col:col + 128], pt[:])

                # ---- merged k setup + block-0 pass 1, software-pipelined by
                # one chunk: per chunk the PE does 4 transposes + 8 matmuls,
                # the DVE does evac(c-1) + casts(c) + ek(c-1), the ACT does the
                # two exps of chunk c-1. All engine budgets sit under the PE's
                # ~1.35us, so the PE never starves after the first chunk. -----
                expT0 = expp.tile([128, NC, BQ], BF16, tag="expT")
                ktiles = {}

                def k_stage(c):
                    kst = stage.tile([128, D], F32, tag="kst")
                    eng = nc.scalar if (c % 2) else nc.sync
                    eng.dma_start(kst[:], k[c * 128:(c + 1) * 128, :])
                    nc.vector.tensor_copy(kr[:, c, 0:256], kst[:, 0:256])
                    nc.vector.tensor_copy(kr[:, c, 260:516], kst[:, 256:512])
                    pt = psT.tile([128, DC, 128], BF16, tag="psT")
                    ksl = [
                        kr[:, c, 0:128], kr[:, c, 128:256],
                        kr[:, c, 260:388], kr[:, c, 388:516],
                    ]
                    for dc in range(DC):
                        nc.tensor.transpose(pt[:, dc, :], ksl[dc], identf[:])
                    ktiles[c] = pt

                def k_evac(c):
                    nc.vector.tensor_copy(
                        kTr[:, :, c * 128:(c + 1) * 128], ktiles.pop(c)[:]
                    )

                def p1_blk0(c):
                    ps_s = psS.tile([128, BQ], F32, tag="psS")
                    for dc in range(DC):
                        nc.tensor.matmul(
                            ps_s[:, 0:257],
                            kTr[:, dc, c * 128:(c + 1) * 128],
                            qsT0[:, dc, 0:257],
                            start=(dc == 0), stop=(dc == DC - 1),
                        )
                    nc.vector.tensor_copy(ek_sb[:, c:c + 1], ps_s[:, 256:257])
                    nc.scalar.activation(
                        expT0[:, c, 0:256], ps_s[:, 0:256], AF.Exp,
                        bias=ek_sb[:, c:c + 1], scale=1.0,
                    )
                    ps_s2 = psS.tile([128, BQ], F32, tag="psS")
                    for dc in range(DC):
                        nc.tensor.matmul(
                            ps_s2[:, 0:256],
                            kTr[:, dc, c * 128:(c + 1) * 128],
                            qsT1[:, dc, 0:256],
                            start=(dc == 0), stop=(dc == DC - 1),
                        )
                    nc.scalar.activation(
                        expT0[:, c, 256:512], ps_s2[:, 0:256], AF.Exp,
                        bias=ek_sb[:, c:c + 1], scale=1.0,
                    )

                k_stage(0)
                for c in range(NC):
                    if c + 1 < NC:
                        k_stage(c + 1)
                    k_evac(c)
                    p1_blk0(c)

                # ---- main loop over q blocks ----------------------------------
                for blk in range(NBLK):
                    i0 = blk * BQ
                    qst = qst_next
                    if blk + 1 < NBLK:
                        qst_next = qstp.tile([128, NIH, D], F32, tag="qst")
                        nc.sync.dma_start(
                            qst_next[:],
                            q[i0 + BQ:i0 + 2 * BQ, :].rearrange(
                                "(t p) d -> p t d", p=128
                            ),
                        )
                    if blk == 0:
                        expT = expT0
                    else:
                        # qsb = bf16(q * w3), then transpose to d-major qsT
                        qsb = qpool.tile([128, NIH, D], BF16, tag="qsb")
                        for t in range(NIH):
                            nc.vector.tensor_mul(
                                qsb[:, t, :], qst[:, t, :], w3sb[:]
                            )
                        qsTn = qpool.tile([128, DC, BQ], BF16, tag="qsTn")
                        for t in range(NIH):
                            pt = psT.tile([128, DC, 128], BF16, tag="psT")
                            for dc in range(DC):
                                nc.tensor.transpose(
                                    pt[:, dc, :],
                                    qsb[:, t, dc * 128:(dc + 1) * 128],
                                    identf[:],
                                )
                            nc.vector.tensor_copy(
                                qsTn[:, :, t * 128:(t + 1) * 128], pt[:]
                            )

                        # pass 1: S^T = kT.T @ qsT chunk by chunk; exp into expT
                        expT = expp.tile([128, NC, BQ], BF16, tag="expT")
                        for c in range(NC):
                            ps_s = psS.tile([128, BQ], F32, tag="psS")
                            for dc in range(DC):
                                nc.tensor.matmul(
                                    ps_s[:],
                                    kTr[:, dc, c * 128:(c + 1) * 128],
                                    qsTn[:, dc, :],
                                    start=(dc == 0),
                                    stop=(dc == DC - 1),
                                )
                            nc.scalar.activation(
                                expT[:, c, :], ps_s[:], AF.Exp,
                                bias=ek_sb[:, c:c + 1], scale=1.0,
                            )

                    # pass 2: AV accumulation per 128-row output slice
                    for ih in range(NIH):
                        pA = psO.tile([128, 257], F32, tag="pA")
                        pB = psO.tile([128, 256], F32, tag="pB")
                        for c in range(NC):
                            lhsT = expT[:, c, ih * 128:(ih + 1) * 128]
                            nc.tensor.matmul(
                                pA[:], lhsT, kr[:, c, 0:257],
                                start=(c == 0), stop=(c == NC - 1),
                            )
                            nc.tensor.matmul(
                                pB[:], lhsT, kr[:, c, 260:516],
                                start=(c == 0), stop=(c == NC - 1),
                            )
                        rec = outp.tile([128, 1], F32, tag="rec")
                        nc.vector.reciprocal(rec[:], pA[:, 256:257])
                        osb = outp.tile([128, 512], F32, tag="osb")
                        nc.scalar.activation(
                            osb[:, 0:256], pA[:, 0:256], AF.Copy, scale=rec[:]
                        )
                        nc.scalar.activation(
                            osb[:, 256:512], pB[:, 0:256], AF.Copy, scale=rec[:]
                        )
                        nc.sync.dma_start(
                            v[i0 + ih * 128:i0 + (ih + 1) * 128, :], osb[:]
                        )

    split_multi_waits(nc)
    return nc


_NC_CACHE = None


def _get_nc():
    global _NC_CACHE
    if _NC_CACHE is None:
        _NC_CACHE = build_attention_nc()
    return _NC_CACHE


def _host_consts(W):
    w2 = np.ascontiguousarray(W[D:2 * D, 0])
    w3 = np.ascontiguousarray(W[2 * D:3 * D, 0])
    w3bc = np.broadcast_to(w3[None, :], (128, D)).copy()
    # w2 striped the way the d-major transpose lays q out: [p, dc]
    w2c16 = w2.reshape(DC, 128).T.astype(ml_dtypes.bfloat16).copy()
    id16 = np.eye(128, dtype=ml_dtypes.bfloat16)
    return w3bc, w2c16, id16


def run(q, k, W, b, trace=False, **spmd_kwargs):
    nc = _get_nc()
    w3bc, w2c16, id16 = _host_consts(np.asarray(W))
    in_maps = [
        {
            "q": np.ascontiguousarray(q[c]),
            "k": np.ascontiguousarray(k[c]),
            "w3bc": w3bc,
            "w2c16": w2c16,
            "id16": id16,
        }
        for c in range(N_CORES)
    ]
    res = run_bass_kernel_spmd(
        nc, in_maps, list(range(N_CORES)), trace=trace, **spmd_kwargs
    )
    out = np.stack([res.results[c]["v"] for c in range(N_CORES)], axis=0)
    return out, res


def kernel(q, k, W, b):
    out, _ = run(np.asarray(q), np.asarray(k), np.asarray(W), np.asarray(b))
    return out
